# revision 5
# baseline (speedup 1.0000x reference)
"""Trainium kernel for nn_EpsilonState: batched log-amplitude of Gaussian-state
overlaps.

Math: the reference's per-sample pair of 128x128 Pfaffians reduces exactly to a
pair of 32x32 Pfaffians S_s built elementwise from four shared 32x32 matrices
(Z11, Z10, Z01, Z00) sign-modulated by the sample's sigma = sign(zz) vector:

    S_s = -(sig sig^T) . Z11 + i (sig 1^T) . Z10 + i (1 sig^T) . Z01 + Z00
    amp_s = C_sector * Pf(S_s)          (C_sector from shared host setup)
    out_b = log(amp_m + x_b[31] * amp_p)

Device: 8 cores x 16 matrices (8 samples x 2 sectors), each core holds its 16
matrices as a 4x4 grid of 32x32 blocks inside [128,128] fp32 re/im tiles and
runs 15 batched Parlett-Reid elimination steps; the Pfaffian is the product of
pivots, accumulated per-matrix. Row broadcasts use constant selector matmuls
on the tensor engine; all elementwise work uses broadcast APs on DVE.
"""
import os
import numpy as np

import concourse.bass as bass
from concourse import bacc
import concourse.mybir as mybir
import concourse.tile as tile
from concourse.bass_utils import run_bass_kernel_spmd

f32 = mybir.dt.float32
P = 128
n = 32
N = 64
NCORES = 8
CSCALE = 64.0
SHIFT = -51.0
AOT = mybir.AluOpType

LAST_RESULTS = None  # stash of BassKernelResults for test harness introspection


# ----------------------------------------------------------------------------
# host-side shared setup (float64 numpy; depends only on s0, H1, H2)
# ----------------------------------------------------------------------------

def _slog_pf(A):
    A = A.copy()
    m = A.shape[0]
    sign_val = 1.0 + 0j
    logpf = 0.0
    for i in range(m - 2):
        x_ = A[:, i].copy()
        nidx = i + 1
        ar = np.arange(m)
        xn = x_[nidx]
        x_[ar <= nidx] = 0
        sigma = np.vdot(x_, x_)
        norm_x = np.sqrt(xn.conj() * xn + sigma)
        phase = 1.0 if xn == 0 else xn / np.abs(xn)
        vn = xn + phase * norm_x
        alpha = -phase * norm_x
        v = x_.copy()
        v[nidx] = vn
        if sigma == 0:
            v = np.zeros_like(x_)
            tau = 0
            alpha = xn
        else:
            v = v / np.linalg.norm(v)
            tau = 2
        w = tau * (A @ v.conj())
        A = A + np.outer(v, w) - np.outer(w, v)
        logpf += np.log(np.abs(1 - tau)) + (np.log(np.abs(-alpha)) if i % 2 == 0 else 0.0)
        sign_val *= ((1 - tau) / np.abs(1 - tau)) * ((-alpha / np.abs(-alpha)) if i % 2 == 0 else 1.0)
    logpf += np.log(np.abs(A[m - 2, m - 1]))
    sign_val *= A[m - 2, m - 1] / np.abs(A[m - 2, m - 1])
    return sign_val, logpf


def _gen_v(zz, PX):
    sgn = np.sign(zz).astype(np.float64).copy()
    sgn[-1] = -PX * sgn[-1]
    norm = 1 / np.sqrt(2.0)
    v = np.zeros((N, n), dtype=np.complex128)
    for k in range(n):
        v[2 * k + 1, k] = -1j * sgn[k] * norm
        v[(2 * k + 2) % N, k] = norm
    return v


def _gf2(L, R):
    M = L.conj().T @ R
    X = np.linalg.solve(M, L.conj().T)
    return np.eye(N) - 2 * (R @ X)


def _logeta_g_expH(H):
    Hh = 1j * (H - H.T) / 2
    e, v = np.linalg.eigh(Hh)
    green = np.real(v @ np.diag(1j * np.tan(e / 2)) @ v.conj().T)
    e_pos = e[: N // 2]
    logeta = np.sum(np.log(np.cos(e_pos / 2).astype(np.complex128)))
    expH = v @ np.diag(np.exp(-1j * e)) @ v.conj().T
    return logeta, green, expH


def _plus_state():
    st = np.zeros((N, n), dtype=np.complex128)
    for k in range(n):
        st[2 * k, k] = -1j / np.sqrt(2)
        st[2 * k + 1, k] = 1 / np.sqrt(2)
    return st


def _minus_state():
    st = np.zeros((N, n), dtype=np.complex128)
    for k in range(n):
        st[2 * k, k] = (1j if k == n - 1 else -1j) / np.sqrt(2)
        st[2 * k + 1, k] = 1 / np.sqrt(2)
    return st


def _log_eta_prop(G1, G2, l1, l2):
    A = (G1 - G1.T) * 0.5
    D = (G2 - G2.T) * 0.5
    pfmat = np.block([[A, -np.eye(N)], [np.eye(N), D]])
    sign_pref = (-1) ** (N // 2)
    s, l = _slog_pf(pfmat)
    return l1 + l2 + np.log(sign_pref * s) + l


def _sector_setup(R, Ghz, logeta_Ghz, PX):
    A = (Ghz - Ghz.T) * 0.5
    Ea = np.zeros((N, n))
    Eb = np.zeros((N, n))
    for k in range(n):
        Ea[2 * k + 1, k] = 1 / np.sqrt(2.0)
        Eb[(2 * k + 2) % N, k] = 1 / np.sqrt(2.0)
    m1 = Ea.T @ R
    m0 = Eb.T @ R
    F11 = R.T @ A @ R
    F11inv = np.linalg.inv(F11)
    P1 = m1.T + R.T @ A @ Ea
    P0 = m0.T + R.T @ A @ Eb
    q11 = Ea.T @ A @ Ea
    q12 = Ea.T @ A @ Eb
    q21 = Eb.T @ A @ Ea
    q22 = Eb.T @ A @ Eb
    Z11 = q11 + P1.T @ F11inv @ P1
    Z10 = q12 + P1.T @ F11inv @ P0
    Z01 = q21 + P0.T @ F11inv @ P1
    Z00 = q22 + P0.T @ F11inv @ P0
    Ainv = np.linalg.inv(A)
    sA, lA = _slog_pf(A)
    sAi, lAi = _slog_pf(Ainv)
    sF, lF = _slog_pf(F11)
    # det(M) is the same for every valid sigma (parity constrained); use x=ones
    xr = np.ones(n)
    zzr = xr * np.roll(xr, -1)
    sig = np.sign(zzr)
    sig[-1] *= -PX
    Ls = Ea * (-1j * sig)[None, :] + Eb
    detM = np.linalg.det(Ls.conj().T @ R)
    logC = (logeta_Ghz + np.log(sA) + lA + np.log(sAi) + lAi
            + np.log(sF) + lF - np.log(detM))
    return dict(Z11=Z11, Z10=Z10, Z01=Z01, Z00=Z00, logC=logC)


_setup_cache = {}
_nc_cache = None


def _shared_setup(s0, H1, H2):
    key = (s0.tobytes(), H1.tobytes(), H2.tobytes())
    if key in _setup_cache:
        return _setup_cache[key]
    ps, ms = _plus_state(), _minus_state()
    zz0 = s0 * np.roll(s0, -1)
    v_plus = _gen_v(zz0, 1)
    v_minus = _gen_v(zz0, -1)
    Gz_plus = _gf2(v_plus, v_plus)
    Gz_minus = _gf2(v_minus, v_minus)
    le_p, G_p, expH_p = _logeta_g_expH(H1)
    le_m, G_m, expH_m = _logeta_g_expH(H2)
    Ghz_plus = _gf2(v_plus, expH_p @ v_plus)
    Ghz_minus = _gf2(v_minus, expH_m @ v_minus)
    logeta_Ghz_plus = _log_eta_prop(G_p, Gz_plus, le_p, 0.0)
    logeta_Ghz_minus = _log_eta_prop(G_m, Gz_minus, le_m, 0.0)
    sp = _sector_setup(ps, Ghz_plus, logeta_Ghz_plus, 1)
    sm = _sector_setup(ms, Ghz_minus, logeta_Ghz_minus, -1)
    K_p = np.exp(sp['logC'] - 16 * np.log(CSCALE) - SHIFT)
    K_m = np.exp(sm['logC'] - 16 * np.log(CSCALE) - SHIFT)

    # z-cat planes [8,128,128]: sector by g parity (even g: plus, odd: minus)
    zplanes = np.zeros((8, P, P), np.float32)
    for gi in range(4):
        st = sp if gi % 2 == 0 else sm
        for pl, mat in enumerate([st['Z11'], st['Z10'], st['Z01'], st['Z00']]):
            blk = (mat * CSCALE).astype(np.complex64)
            for mi in range(4):
                zplanes[2 * pl, mi * 32:mi * 32 + 32, gi * 32:gi * 32 + 32] = blk.real
                zplanes[2 * pl + 1, mi * 32:mi * 32 + 32, gi * 32:gi * 32 + 32] = blk.imag

    sel = np.zeros((31, P, P), np.float32)
    for j in range(31):
        for p in range(P):
            sel[j, (p // 32) * 32 + j, p] = 1.0

    dmask = np.zeros((16, P, 32), np.float32)
    for s in range(16):
        k = 2 * s
        for p in range(P):
            c = p % 32
            if c >= k + 2:
                dmask[s, p, c] = 1.0

    res = dict(zplanes=zplanes, sel=sel, dmask=dmask, K_p=K_p, K_m=K_m)
    _setup_cache[key] = res
    return res


# ----------------------------------------------------------------------------
# device program
# ----------------------------------------------------------------------------

def _build_nc(nsteps=15, dump=False):
    global _nc_cache
    if _nc_cache is not None and nsteps == 15 and not dump:
        return _nc_cache
    nc = bacc.Bacc()
    zcat_d = nc.dram_tensor("zcat", [8, P, P], f32, kind="ExternalInput")
    sel_d = nc.dram_tensor("sel", [31, P, P], f32, kind="ExternalInput")
    dmask_d = nc.dram_tensor("dmask", [16, P, 32], f32, kind="ExternalInput")
    sigc_d = nc.dram_tensor("sigc", [P, 4], f32, kind="ExternalInput")
    sigf_d = nc.dram_tensor("sigf", [P, P], f32, kind="ExternalInput")
    kre_d = nc.dram_tensor("kre", [P, 4], f32, kind="ExternalInput")
    kim_d = nc.dram_tensor("kim", [P, 4], f32, kind="ExternalInput")
    out_d = nc.dram_tensor("out", [P, 4], f32, kind="ExternalOutput")

    with tile.TileContext(nc) as tc:
        with tc.tile_pool(name="const", bufs=1) as cpool, \
             tc.tile_pool(name="state", bufs=1) as spool, \
             tc.tile_pool(name="temps", bufs=2) as tpool, \
             tc.tile_pool(name="psum", bufs=2, space="PSUM") as ppool:

            zc = cpool.tile([P, 8, P], f32, tag="zc")
            for i in range(8):
                nc.sync.dma_start(zc[:, i, :], zcat_d[i])
            selt = cpool.tile([P, 31, P], f32, tag="selt")
            for j in range(31):
                nc.sync.dma_start(selt[:, j, :], sel_d[j])
            dm = cpool.tile([P, 16, 32], f32, tag="dm")
            for i in range(16):
                nc.sync.dma_start(dm[:, i, :], dmask_d[i])
            sigc = cpool.tile([P, 4], f32, tag="sigc")
            nc.sync.dma_start(sigc[:], sigc_d[:])
            sigf = cpool.tile([P, P], f32, tag="sigf")
            nc.sync.dma_start(sigf[:], sigf_d[:])
            kre = cpool.tile([P, 4], f32, tag="kre")
            nc.sync.dma_start(kre[:], kre_d[:])
            kim = cpool.tile([P, 4], f32, tag="kim")
            nc.sync.dma_start(kim[:], kim_d[:])

            Sre = spool.tile([P, 4, 32], f32, tag="Sre")
            Sim = spool.tile([P, 4, 32], f32, tag="Sim")
            prod_re = spool.tile([P, 4], f32, tag="prodre")
            prod_im = spool.tile([P, 4], f32, tag="prodim")
            nc.vector.memset(prod_re[:], 1.0)
            nc.vector.memset(prod_im[:], 0.0)

            def z(i):
                return zc[:, i, :].rearrange("p (g c) -> p g c", g=4)

            sigf_v = sigf[:].rearrange("p (g c) -> p g c", g=4)
            sigc_b = sigc[:, :, None].broadcast_to([P, 4, 32])

            # ---- build S = -so*Z11 + i*sig_r*Z10 + i*sig_c*Z01 + Z00 ----
            so = tpool.tile([P, 4, 32], f32, tag="so")
            nc.vector.tensor_mul(so[:], sigf_v, sigc_b)
            t0 = tpool.tile([P, 4, 32], f32, tag="t0")
            # S_re = Z00re - so*Z11re - sig_r*Z10im - sig_c*Z01im
            nc.vector.tensor_mul(t0[:], so[:], z(0))
            nc.vector.tensor_sub(Sre[:], z(6), t0[:])
            nc.vector.tensor_mul(t0[:], z(3), sigc_b)
            nc.vector.tensor_sub(Sre[:], Sre[:], t0[:])
            nc.vector.tensor_mul(t0[:], z(5), sigf_v)
            nc.vector.tensor_sub(Sre[:], Sre[:], t0[:])
            # S_im = Z00im - so*Z11im + sig_r*Z10re + sig_c*Z01re
            nc.vector.tensor_mul(t0[:], so[:], z(1))
            nc.vector.tensor_sub(Sim[:], z(7), t0[:])
            nc.vector.tensor_mul(t0[:], z(2), sigc_b)
            nc.vector.tensor_add(Sim[:], Sim[:], t0[:])
            nc.vector.tensor_mul(t0[:], z(4), sigf_v)
            nc.vector.tensor_add(Sim[:], Sim[:], t0[:])

            Sre_f = Sre[:].rearrange("p g c -> p (g c)")
            Sim_f = Sim[:].rearrange("p g c -> p (g c)")

            def pivot_accum(ar_ps, ai_ps, col):
                """multiply running pivot product by element [*, col] of the
                broadcast row (ar/ai in PSUM, viewed (p,g,c))."""
                pvr = tpool.tile([P, 4], f32, tag="pvr")
                pvi = tpool.tile([P, 4], f32, tag="pvi")
                nc.vector.tensor_copy(pvr[:], ar_ps[:, :, col])
                nc.vector.tensor_copy(pvi[:], ai_ps[:, :, col])
                t1 = tpool.tile([P, 4], f32, tag="pt1")
                t2 = tpool.tile([P, 4], f32, tag="pt2")
                t3 = tpool.tile([P, 4], f32, tag="pt3")
                t4 = tpool.tile([P, 4], f32, tag="pt4")
                nc.vector.tensor_mul(t1[:], prod_re[:], pvr[:])
                nc.vector.tensor_mul(t2[:], prod_im[:], pvi[:])
                nc.vector.tensor_mul(t3[:], prod_re[:], pvi[:])
                nc.vector.tensor_mul(t4[:], prod_im[:], pvr[:])
                nc.vector.tensor_sub(prod_re[:], t1[:], t2[:])
                nc.vector.tensor_add(prod_im[:], t3[:], t4[:])
                return pvr, pvi

            for s in range(nsteps):
                k = 2 * s
                # row broadcasts via selector matmuls (PE)
                ar = ppool.tile([P, 4, 32], f32, tag="ar")
                ai = ppool.tile([P, 4, 32], f32, tag="ai")
                br = ppool.tile([P, 4, 32], f32, tag="br")
                bi = ppool.tile([P, 4, 32], f32, tag="bi")
                ar_f = ar[:].rearrange("p g c -> p (g c)")
                ai_f = ai[:].rearrange("p g c -> p (g c)")
                br_f = br[:].rearrange("p g c -> p (g c)")
                bi_f = bi[:].rearrange("p g c -> p (g c)")
                nc.tensor.matmul(ar_f, selt[:, k, :], Sre_f, start=True, stop=True)
                nc.tensor.matmul(ai_f, selt[:, k, :], Sim_f, start=True, stop=True)
                nc.tensor.matmul(br_f, selt[:, k + 1, :], Sre_f, start=True, stop=True)
                nc.tensor.matmul(bi_f, selt[:, k + 1, :], Sim_f, start=True, stop=True)

                # pivot = S[k, k+1] = ar[:, :, k+1]; accumulate into prod
                pvr, pvi = pivot_accum(ar, ai, k + 1)

                # 1/pivot
                dt1 = tpool.tile([P, 4], f32, tag="dt1")
                dt2 = tpool.tile([P, 4], f32, tag="dt2")
                den = tpool.tile([P, 4], f32, tag="den")
                nc.vector.tensor_mul(dt1[:], pvr[:], pvr[:])
                nc.vector.tensor_mul(dt2[:], pvi[:], pvi[:])
                nc.vector.tensor_add(den[:], dt1[:], dt2[:])
                rec = tpool.tile([P, 4], f32, tag="rec")
                nc.vector.reciprocal(rec[:], den[:])
                ivr = tpool.tile([P, 4], f32, tag="ivr")
                ivi = tpool.tile([P, 4], f32, tag="ivi")
                nc.vector.tensor_mul(ivr[:], pvr[:], rec[:])
                nc.vector.tensor_mul(ivi[:], pvi[:], rec[:])

                # r-side vectors extracted FROM the broadcast rows (masked
                # diagonal reduce) so both outer-product factors come from the
                # same row data; mixing row/col sources compounds antisymmetry
                # roundoff exponentially.
                acr = tpool.tile([P, 4], f32, tag="acr")   # = -a_r masked
                aci = tpool.tile([P, 4], f32, tag="aci")
                bcr = tpool.tile([P, 4], f32, tag="bcr")   # = -b_r masked
                bci = tpool.tile([P, 4], f32, tag="bci")
                dmk = dm[:, s, None, :].broadcast_to([P, 4, 32])
                ext = tpool.tile([P, 4, 32], f32, tag="ext")
                for (rowt, dst) in ((ar, acr), (ai, aci), (br, bcr), (bi, bci)):
                    nc.vector.tensor_mul(ext[:], rowt[:], dmk)
                    nc.vector.tensor_reduce(dst[:], ext[:],
                                            axis=mybir.AxisListType.X, op=AOT.add)

                # u2 = (-b)*conj(piv)/|piv|^2 ; w2 = (-a)*conj(piv)/|piv|^2
                ur = tpool.tile([P, 4], f32, tag="ur")
                ui = tpool.tile([P, 4], f32, tag="ui")
                wr = tpool.tile([P, 4], f32, tag="wr")
                wi = tpool.tile([P, 4], f32, tag="wi")
                nc.vector.tensor_mul(dt1[:], bcr[:], ivr[:])
                nc.vector.tensor_mul(dt2[:], bci[:], ivi[:])
                nc.vector.tensor_add(ur[:], dt1[:], dt2[:])
                nc.vector.tensor_mul(dt1[:], bci[:], ivr[:])
                nc.vector.tensor_mul(dt2[:], bcr[:], ivi[:])
                nc.vector.tensor_sub(ui[:], dt1[:], dt2[:])
                nc.vector.tensor_mul(dt1[:], acr[:], ivr[:])
                nc.vector.tensor_mul(dt2[:], aci[:], ivi[:])
                nc.vector.tensor_add(wr[:], dt1[:], dt2[:])
                nc.vector.tensor_mul(dt1[:], aci[:], ivr[:])
                nc.vector.tensor_mul(dt2[:], acr[:], ivi[:])
                nc.vector.tensor_sub(wi[:], dt1[:], dt2[:])

                # U = +u2 (x) arow - w2 (x) brow   (complex)
                pt = tpool.tile([P, 4, 32], f32, tag="pt")
                for (cvec, rowt, sign, Sdst) in (
                        (ur, ar, 1.0, Sre), (ui, ai, -1.0, Sre),
                        (wr, br, -1.0, Sre), (wi, bi, 1.0, Sre),
                        (ur, ai, 1.0, Sim), (ui, ar, 1.0, Sim),
                        (wr, bi, -1.0, Sim), (wi, br, -1.0, Sim)):
                    nc.vector.tensor_mul(
                        pt[:], rowt[:],
                        cvec[:, :, None].broadcast_to([P, 4, 32]))
                    nc.vector.scalar_tensor_tensor(
                        out=Sdst[:], in0=pt[:], scalar=sign, in1=Sdst[:],
                        op0=AOT.mult, op1=AOT.add)

            if dump:
                dump_sre = nc.dram_tensor("dump_sre", [P, P], f32, kind="ExternalOutput")
                dump_sim = nc.dram_tensor("dump_sim", [P, P], f32, kind="ExternalOutput")
                dump_pre = nc.dram_tensor("dump_pre", [P, 4], f32, kind="ExternalOutput")
                dump_pim = nc.dram_tensor("dump_pim", [P, 4], f32, kind="ExternalOutput")
                nc.sync.dma_start(dump_sre[:], Sre_f)
                nc.sync.dma_start(dump_sim[:], Sim_f)
                nc.sync.dma_start(dump_pre[:], prod_re[:])
                nc.sync.dma_start(dump_pim[:], prod_im[:])
            # final pivot: S[30, 31]
            ar = ppool.tile([P, 4, 32], f32, tag="ar")
            ai = ppool.tile([P, 4, 32], f32, tag="ai")
            nc.tensor.matmul(ar[:].rearrange("p g c -> p (g c)"), selt[:, 30, :],
                             Sre_f, start=True, stop=True)
            nc.tensor.matmul(ai[:].rearrange("p g c -> p (g c)"), selt[:, 30, :],
                             Sim_f, start=True, stop=True)
            pivot_accum(ar, ai, 31)

            # E = K * prod ; pair-sum sectors; emit [re0, im0, re1, im1]
            er = tpool.tile([P, 4], f32, tag="er")
            ei = tpool.tile([P, 4], f32, tag="ei")
            t1 = tpool.tile([P, 4], f32, tag="ft1")
            t2 = tpool.tile([P, 4], f32, tag="ft2")
            nc.vector.tensor_mul(t1[:], prod_re[:], kre[:])
            nc.vector.tensor_mul(t2[:], prod_im[:], kim[:])
            nc.vector.tensor_sub(er[:], t1[:], t2[:])
            nc.vector.tensor_mul(t1[:], prod_re[:], kim[:])
            nc.vector.tensor_mul(t2[:], prod_im[:], kre[:])
            nc.vector.tensor_add(ei[:], t1[:], t2[:])
            outt = tpool.tile([P, 2, 2], f32, tag="outt")
            er_v = er[:].rearrange("p (j t) -> p j t", t=2)
            ei_v = ei[:].rearrange("p (j t) -> p j t", t=2)
            nc.vector.tensor_add(outt[:, :, 0], er_v[:, :, 0], er_v[:, :, 1])
            nc.vector.tensor_add(outt[:, :, 1], ei_v[:, :, 0], ei_v[:, :, 1])
            nc.sync.dma_start(out_d[:], outt[:].rearrange("p j t -> p (j t)"))

    nc.compile()
    if nsteps == 15 and not dump:
        _nc_cache = nc
    return nc


# ----------------------------------------------------------------------------
# entry point
# ----------------------------------------------------------------------------

def kernel(x, s0, H1, H2):
    global LAST_RESULTS
    x64 = np.asarray(x, dtype=np.float64)
    s064 = np.asarray(s0, dtype=np.float64)
    H164 = np.asarray(H1, dtype=np.float64)
    H264 = np.asarray(H2, dtype=np.float64)
    B = x64.shape[0]
    assert B == 64 and x64.shape[1] == n

    st = _shared_setup(s064, H164, H264)
    nc = _build_nc()

    zz = x64 * np.roll(x64, -1, axis=1)          # [64, 32]
    sgn = np.sign(zz)

    in_maps = []
    for c in range(NCORES):
        sigc = np.zeros((P, 4), np.float32)
        sigf = np.zeros((P, P), np.float32)
        kre = np.zeros((P, 4), np.float32)
        kim = np.zeros((P, 4), np.float32)
        for mi in range(4):
            for gi in range(4):
                samp = c * 8 + mi * 2 + gi // 2
                plus = (gi % 2 == 0)
                sig = sgn[samp].copy()
                sig[-1] *= -1.0 if plus else 1.0
                sigc[mi * 32:mi * 32 + 32, gi] = sig
                sigf[mi * 32:mi * 32 + 32, gi * 32:gi * 32 + 32] = sig[None, :]
                K = st['K_p'] * (x64[samp, -1] * s064[-1]) if plus else st['K_m']
                kre[mi * 32:mi * 32 + 32, gi] = np.float32(K.real)
                kim[mi * 32:mi * 32 + 32, gi] = np.float32(K.imag)
        in_maps.append(dict(zcat=st['zplanes'], sel=st['sel'], dmask=st['dmask'],
                            sigc=sigc, sigf=sigf, kre=kre, kim=kim))

    trace = bool(int(os.environ.get("PFK_TRACE", "0")))
    res = run_bass_kernel_spmd(nc, in_maps, core_ids=list(range(NCORES)),
                               trace=trace)
    LAST_RESULTS = res

    out = np.zeros(B, dtype=np.complex128)
    for c in range(NCORES):
        o = res.results[c]["out"]
        for mi in range(4):
            for j in range(2):
                zv = complex(o[mi * 32, 2 * j], o[mi * 32, 2 * j + 1])
                out[c * 8 + mi * 2 + j] = np.log(zv) + SHIFT
    return out
